# revision 9
# baseline (speedup 1.0000x reference)
"""EfficientAttention (linear attention) Trainium2 Bass kernel.

Computes, per batch b:
    q_n = softmax(q[b], axis=-1)        # over feature dim D=64
    k_n = softmax(k[b], axis=-1)
    ctx = k_n^T @ v[b]                  # [D, D]
    out[b] = q_n @ ctx                  # [N, D]

Sharding: batch dim (32) split across 8 cores, 4 batches per core.

Design notes (per core):
- I/O dtype is fp16: the host casts q/k/v fp32 -> fp16 before upload and
  upcasts o fp16 -> fp32 after download (rel err ~3e-4 vs the 2e-2 gate).
  This halves HBM traffic (67.1 MB -> 33.6 MB per core), which is the
  roofline for this memory-bound problem (~358 GB/s/core).
- DMA: 512 KB loads/stores ([128 partitions, 4 KB contiguous per partition];
  rows interleaved so partition p holds rows n0+32p .. n0+32p+31).
- fp32 matmuls on the PE run as two half-speed passes (fp32_mode=LOW_HIGH);
  fp16 runs single-pass with fast-weight-load. All matmul inputs are fp16:
  raw q/k/v arrive fp16 from HBM, ACT exp writes fp16, the DVE normalize
  writes fp16. PSUM accumulation stays fp32; k row-sums are fp32.
- K/V pass: exp(k) on ACT, row-sums + reciprocal + scale on DVE,
  PE accumulates ctx[64,64] over N.
- ctx epilogue: block-diagonal stacked ctxa [128, 130] fp16
  (rows 0:64 = [ctx | 1 | 0], rows 64:128 = [0 | ctx | 1]) so one K=128
  matmul computes two packed row-tiles (cols 0:65 and 65:130) with a
  single full row group. (Matmuls with alternating row groups writing one
  PSUM bank lock up the device - found by bisection.)
- Q pass: PE-transpose raw q pairs [128, 2x64] -> PSUM [128,128] (feature
  dim onto partitions, two row-tiles stacked), ACT exp PSUM->SBUF (fused
  evict + exp + fp16 cast), one matmul per pair against ctxa -> [128,130]
  (col 64/129 = row-sums via the ones columns), DVE reciprocal + multiply
  -> natural-layout fp32 output.
- Batch b's q-pass is interleaved with batch b+1's k/v-pass to keep the
  PE dense (HAM stays un-throttled) and the DMA queues evenly loaded.
"""

import numpy as np

import concourse.bass as bass
import concourse.mybir as mybir
import concourse.tile as tile
from concourse import bacc
from concourse.bass_utils import run_bass_kernel_spmd

B, N, D = 32, 16384, 64
NCORES = 8
BPC = B // NCORES  # batches per core
LOAD = 4096  # rows per DMA (1 MB fp32)
LT = LOAD // 128  # row-tile slots per load (32)
NBLK = N // LOAD  # load blocks per batch (4)
F32 = mybir.dt.float32
F16 = mybir.dt.float16
EXP = mybir.ActivationFunctionType.Exp


def build_bass():
    nc = bacc.Bacc("TRN2", target_bir_lowering=False, debug=False)
    q = nc.dram_tensor("q", [BPC, N, D], F16, kind="ExternalInput").ap()
    k = nc.dram_tensor("k", [BPC, N, D], F16, kind="ExternalInput").ap()
    v = nc.dram_tensor("v", [BPC, N, D], F16, kind="ExternalInput").ap()
    o = nc.dram_tensor("o", [BPC, N, D], F16, kind="ExternalOutput").ap()

    def blk(t, b, n0):
        return t[b, n0 : n0 + LOAD, :].rearrange("(p t) d -> p t d", p=128)

    with tile.TileContext(nc) as tc:
        with (
            tc.tile_pool(name="consts", bufs=1) as consts,
            tc.tile_pool(name="io", bufs=2) as io,
            tc.tile_pool(name="work", bufs=3) as work,
            tc.tile_pool(name="ctxp", bufs=2) as ctxp,
            tc.tile_pool(name="ps_t", bufs=2, space="PSUM") as ps_t,
            tc.tile_pool(name="ps_o", bufs=4, space="PSUM") as ps_o,
            tc.tile_pool(name="ps_c", bufs=2, space="PSUM") as ps_c,
        ):
            from concourse.masks import make_identity

            ident = consts.tile([128, 128], F16)
            make_identity(nc, ident)

            ctx_ps = {}

            def emit_kv_block(b, i):
                n0 = i * LOAD
                k_sb = io.tile([128, LT, 64], F16, tag="k_sb", bufs=4)
                v_sb = io.tile([128, LT, 64], F16, tag="v_sb", bufs=4)
                nc.sync.dma_start(out=k_sb, in_=blk(k, b, n0))
                nc.sync.dma_start(out=v_sb, in_=blk(v, b, n0))
                ek = work.tile([128, LT, 64], F32, tag="ek")
                nc.scalar.activation(ek, k_sb, EXP)
                ks = work.tile([128, LT, 1], F32, tag="ks")
                nc.vector.reduce_sum(out=ks, in_=ek, axis=mybir.AxisListType.X)
                ksr = work.tile([128, LT, 1], F32, tag="ksr")
                nc.vector.reciprocal(ksr, ks)
                ekn = work.tile([128, LT, 64], F16, tag="ekn", bufs=4)
                nc.gpsimd.tensor_mul(ekn, ek, ksr[:].to_broadcast((128, LT, 64)))
                for t in range(LT):
                    nc.tensor.matmul(
                        ctx_ps[b],
                        ekn[:, t, :],
                        v_sb[:, t, :],
                        start=(i == 0 and t == 0),
                        stop=(i == NBLK - 1 and t == LT - 1),
                    )

            def emit_ctx_epilogue(b):
                ctxa = ctxp.tile([128, 130], F16, tag="ctxa")
                nc.vector.memset(ctxa, 0.0)
                nc.vector.tensor_copy(ctxa[0:64, 0:64], ctx_ps[b])
                nc.vector.memset(ctxa[0:64, 64:65], 1.0)
                nc.scalar.dma_start(out=ctxa[64:128, 65:130], in_=ctxa[0:64, 0:65])
                return ctxa

            def load_q_block(b, i):
                q_sb = io.tile([128, LT, 64], F16, tag="q_sb", bufs=4, name="q_sb")
                nc.sync.dma_start(out=q_sb, in_=blk(q, b, i * LOAD))
                return q_sb

            def emit_q_block(b, i, ctxa, q_sb=None, split_store=False):
                n0 = i * LOAD
                if q_sb is None:
                    q_sb = load_q_block(b, i)
                out_sb = io.tile([128, LT, 64], F16, tag="out_sb", bufs=3)
                for c in range(LT // 8):  # 1024-row compute chunks
                    tp_ps = ps_t.tile([128, 4, 128], F16, tag="tp_ps")
                    for u in range(4):
                        s0 = 8 * c + 2 * u
                        nc.tensor.transpose(
                            tp_ps[:, u, :],
                            q_sb[:, s0 : s0 + 2, :].rearrange("p t d -> p (t d)"),
                            ident,
                        )
                    eqT = work.tile([128, 4, 128], F16, tag="eqT", bufs=8)
                    nc.scalar.activation(eqT, tp_ps, EXP)
                    for g in range(2):
                        o_ps = ps_o.tile([128, 2, 132], F32, tag="o_ps")
                        for s in range(2):
                            nc.tensor.matmul(
                                o_ps[:, s, 0:130],
                                eqT[:, 2 * g + s, :],
                                ctxa,
                                start=True,
                                stop=True,
                            )
                        opb = o_ps[:]
                        pdim = opb.ap[0]
                        sstep = opb.ap[1][0]  # slot stride (132)
                        cstep = opb.ap[2][0]  # col stride (1)
                        r_sb = work.tile([128, 2, 2, 1], F32, tag="r_sb")
                        rs_ap = bass.AP(
                            tensor=opb.tensor,
                            offset=opb.offset + 64 * cstep,
                            ap=[pdim, [sstep, 2], [65 * cstep, 2], [cstep, 1]],
                        )
                        nc.vector.reciprocal(r_sb, rs_ap)
                        vals_ap = bass.AP(
                            tensor=opb.tensor,
                            offset=opb.offset,
                            ap=[pdim, [sstep, 2], [65 * cstep, 2], [cstep, 64]],
                        )
                        t0 = 8 * c + 4 * g
                        out_view = out_sb[:, t0 : t0 + 4, :].rearrange(
                            "p (s t) d -> p s t d", s=2
                        )
                        nc.vector.tensor_mul(
                            out_view,
                            vals_ap,
                            r_sb[:].to_broadcast((128, 2, 2, 64)),
                        )
                    if split_store:
                        nc.scalar.dma_start(
                            out=blk(o, b, n0)[:, 8 * c : 8 * c + 8, :],
                            in_=out_sb[:, 8 * c : 8 * c + 8, :],
                        )
                if not split_store:
                    nc.scalar.dma_start(out=blk(o, b, n0), in_=out_sb)

            # software-pipelined schedule: q-pass(b) interleaved with kv(b+1)
            ctx_ps[0] = ps_c.tile([64, 64], F32, tag="ctx_ps", name="ctx_ps")
            q_pre = [load_q_block(0, 0), load_q_block(0, 1)]
            for i in range(NBLK):
                emit_kv_block(0, i)
            ctxa = emit_ctx_epilogue(0)
            for b in range(BPC):
                if b + 1 < BPC:
                    ctx_ps[b + 1] = ps_c.tile([64, 64], F32, tag="ctx_ps", name="ctx_ps")
                nxt = None
                for i in range(NBLK):
                    # kv(b+1) first so its ctx completes before q(b) drains;
                    # epilogue right after the last kv block
                    if b + 1 < BPC:
                        emit_kv_block(b + 1, i)
                        if i == NBLK - 1:
                            nxt = emit_ctx_epilogue(b + 1)
                    last = b == BPC - 1 and i == NBLK - 1
                    emit_q_block(
                        b, i, ctxa,
                        q_sb=q_pre.pop(0) if (b == 0 and q_pre) else None,
                        split_store=last,
                    )
                if nxt is not None:
                    ctxa = nxt

    nc.compile()
    return nc


_NC_CACHE = None


def kernel(q: np.ndarray, k: np.ndarray, v: np.ndarray) -> np.ndarray:
    global _NC_CACHE
    if _NC_CACHE is None:
        _NC_CACHE = build_bass()
    nc = _NC_CACHE
    q = np.ascontiguousarray(np.asarray(q), dtype=np.float16)
    k = np.ascontiguousarray(np.asarray(k), dtype=np.float16)
    v = np.ascontiguousarray(np.asarray(v), dtype=np.float16)
    in_maps = [
        {
            "q": q[i * BPC : (i + 1) * BPC],
            "k": k[i * BPC : (i + 1) * BPC],
            "v": v[i * BPC : (i + 1) * BPC],
        }
        for i in range(NCORES)
    ]
    res = run_bass_kernel_spmd(nc, in_maps, core_ids=list(range(NCORES)))
    return np.concatenate(
        [res.results[i]["o"] for i in range(NCORES)], axis=0
    ).astype(np.float32)



# revision 16
# speedup vs baseline: 1.0317x; 1.0317x over previous
"""EfficientAttention (linear attention) Trainium2 Bass kernel.

Computes, per batch b:
    q_n = softmax(q[b], axis=-1)        # over feature dim D=64
    k_n = softmax(k[b], axis=-1)
    ctx = k_n^T @ v[b]                  # [D, D]
    out[b] = q_n @ ctx                  # [N, D]

Sharding: batch dim (32) split across 8 cores, 4 batches per core.

Design notes (per core):
- I/O dtype is fp16: the host casts q/k/v fp32 -> fp16 before upload and
  upcasts o fp16 -> fp32 after download (rel err ~1e-3 vs the 2e-2 gate).
  This halves HBM traffic (67.1 MB -> 33.6 MB per core), which is the
  roofline for this memory-bound problem (~358 GB/s/core).
- DMA: 512 KB loads/stores ([128 partitions, 4 KB contiguous per partition];
  rows interleaved so partition p holds rows n0+32p .. n0+32p+31). Loads
  are prefetched two blocks ahead of their consumers.
- K/V pass: exp(k) on ACT (fp16 out), row-sums + reciprocal on DVE,
  normalize on gpsimd (otherwise idle), PE accumulates ctx[64,64] over N.
- ctx epilogue: block-diagonal stacked ctxa [128, 130] fp16
  (rows 0:64 = [ctx | 1 | 0], rows 64:128 = [0 | ctx | 1]) so one K=128
  matmul computes two packed row-tiles (cols 0:65 and 65:130, col 64/129
  = row sums via the ones columns).
- Q pass: PE-transpose raw q pairs [128, 2x64] -> PSUM [128,8,128] fp16,
  ACT exp PSUM->SBUF (fused evict + exp), per-slot matmuls vs ctxa into a
  2-bank PSUM tile [128, 2, 512] with slots padded to uniform strides
  (slot w at bank w//2, offset 132*(w%2)), so the divide is ONE bulk
  reciprocal + ONE [128,2,2,2,64] DVE multiply per 1024 rows instead of
  128 tiny ops per core (walrus emits one LDWEIGHTS per matmul - no
  dedupe - so a separate sums matmul would cost PE 14 us; ones-columns
  are free).
- Batch b's q-pass is interleaved with batch b+1's k/v-pass to keep the
  PE dense, and (Matmuls with alternating row groups writing one PSUM
  bank lock up the device) each matmul writes a full row group.
"""

import numpy as np

import concourse.bass as bass
import concourse.mybir as mybir
import concourse.tile as tile
from concourse import bacc
from concourse.bass_utils import run_bass_kernel_spmd

B, N, D = 32, 16384, 64
NCORES = 8
BPC = B // NCORES  # batches per core
LOAD = 4096  # rows per DMA (512 KB fp16)
LT = LOAD // 128  # row-tile slots per load (32)
NBLK = N // LOAD  # load blocks per batch (4)
F32 = mybir.dt.float32
F16 = mybir.dt.float16
EXP = mybir.ActivationFunctionType.Exp


def build_bass():
    nc = bacc.Bacc("TRN2", target_bir_lowering=False, debug=False)
    q = nc.dram_tensor("q", [BPC, N, D], F16, kind="ExternalInput").ap()
    k = nc.dram_tensor("k", [BPC, N, D], F16, kind="ExternalInput").ap()
    v = nc.dram_tensor("v", [BPC, N, D], F16, kind="ExternalInput").ap()
    o = nc.dram_tensor("o", [BPC, N, D], F16, kind="ExternalOutput").ap()

    def blk(t, b, n0):
        return t[b, n0 : n0 + LOAD, :].rearrange("(p t) d -> p t d", p=128)

    with tile.TileContext(nc) as tc:
        with (
            tc.tile_pool(name="consts", bufs=1) as consts,
            tc.tile_pool(name="io", bufs=2) as io,
            tc.tile_pool(name="work", bufs=3) as work,
            tc.tile_pool(name="ctxp", bufs=2) as ctxp,
            tc.tile_pool(name="ps_t", bufs=2, space="PSUM") as ps_t,
            tc.tile_pool(name="ps_o", bufs=2, space="PSUM") as ps_o,
            tc.tile_pool(name="ps_c", bufs=2, space="PSUM") as ps_c,
        ):
            from concourse.masks import make_identity

            ident = consts.tile([128, 128], F16)
            make_identity(nc, ident)

            ctx_ps = {}
            kv_queue = []
            q_queue = []

            def load_kv_block(b, i):
                n0 = i * LOAD
                k_sb = io.tile([128, LT, 64], F16, tag="k_sb", bufs=6)
                v_sb = io.tile([128, LT, 64], F16, tag="v_sb", bufs=6)
                nc.sync.dma_start(out=k_sb, in_=blk(k, b, n0))
                nc.sync.dma_start(out=v_sb, in_=blk(v, b, n0))
                kv_queue.append((k_sb, v_sb))

            def load_q_block(b, i):
                q_sb = io.tile([128, LT, 64], F16, tag="q_sb", bufs=5, name="q_sb")
                nc.sync.dma_start(out=q_sb, in_=blk(q, b, i * LOAD))
                q_queue.append(q_sb)

            def emit_kv_block(b, i):
                k_sb, v_sb = kv_queue.pop(0)
                ek = work.tile([128, LT, 64], F16, tag="ek", bufs=4)
                nc.scalar.activation(ek, k_sb, EXP)
                ks = work.tile([128, LT, 1], F32, tag="ks")
                nc.vector.reduce_sum(out=ks, in_=ek, axis=mybir.AxisListType.X)
                ksr = work.tile([128, LT, 1], F32, tag="ksr")
                nc.vector.reciprocal(ksr, ks)
                ekn = work.tile([128, LT, 64], F16, tag="ekn", bufs=4)
                nc.gpsimd.tensor_mul(ekn, ek, ksr[:].to_broadcast((128, LT, 64)))
                for t in range(LT):
                    nc.tensor.matmul(
                        ctx_ps[b],
                        ekn[:, t, :],
                        v_sb[:, t, :],
                        start=(i == 0 and t == 0),
                        stop=(i == NBLK - 1 and t == LT - 1),
                    )

            def emit_ctx_epilogue(b):
                ctxa = ctxp.tile([128, 130], F16, tag="ctxa")
                nc.vector.memset(ctxa, 0.0)
                nc.vector.tensor_copy(ctxa[0:64, 0:64], ctx_ps[b])
                nc.vector.memset(ctxa[0:64, 64:65], 1.0)
                nc.scalar.dma_start(out=ctxa[64:128, 65:130], in_=ctxa[0:64, 0:65])
                return ctxa

            def emit_q_block(b, i, ctxa, split_store=False):
                n0 = i * LOAD
                q_sb = q_queue.pop(0)
                out_sb = io.tile([128, LT, 64], F16, tag="out_sb", bufs=4)
                for h in range(2):  # half-block: 16 slots = 2048 rows
                    tp_ps = ps_t.tile([128, 8, 128], F16, tag="tp_ps")
                    for u in range(8):
                        s0 = 16 * h + 2 * u
                        nc.tensor.transpose(
                            tp_ps[:, u, :],
                            q_sb[:, s0 : s0 + 2, :].rearrange("p t d -> p (t d)"),
                            ident,
                        )
                    eqT = work.tile([128, 8, 128], F16, tag="eqT", bufs=6)
                    nc.scalar.activation(eqT, tp_ps, EXP)
                    for g in range(2):
                        # 2-bank PSUM tile; slot w at offset 256*w (uniform
                        # stride across banks, 130 of 256 used) so the
                        # divide APs stay 4D (TENSOR3D encoding limit).
                        o_ps = ps_o.tile([128, 2, 512], F32, tag="o_ps")
                        for w in range(4):
                            u = 4 * g + w
                            nc.tensor.matmul(
                                o_ps[:, w // 2, 256 * (w % 2) : 256 * (w % 2) + 130],
                                eqT[:, u, :],
                                ctxa,
                                start=True,
                                stop=True,
                            )
                        opb = o_ps[:]
                        pdim = opb.ap[0]
                        rsr = work.tile([128, 4, 2, 1], F32, tag="rsr")
                        rs_ap = bass.AP(
                            tensor=opb.tensor,
                            offset=opb.offset + 64,
                            ap=[pdim, [256, 4], [65, 2], [1, 1]],
                        )
                        nc.vector.reciprocal(rsr, rs_ap)
                        vals_ap = bass.AP(
                            tensor=opb.tensor,
                            offset=opb.offset,
                            ap=[pdim, [256, 4], [65, 2], [1, 64]],
                        )
                        t0 = 16 * h + 8 * g
                        out_view = out_sb[:, t0 : t0 + 8, :].rearrange(
                            "p (s t) d -> p s t d", s=4
                        )
                        nc.vector.tensor_mul(
                            out_view,
                            vals_ap,
                            rsr[:].to_broadcast((128, 4, 2, 64)),
                        )
                    if split_store:
                        nc.scalar.dma_start(
                            out=blk(o, b, n0)[:, 16 * h : 16 * h + 16, :],
                            in_=out_sb[:, 16 * h : 16 * h + 16, :],
                        )
                if not split_store:
                    nc.scalar.dma_start(out=blk(o, b, n0), in_=out_sb)

            def nxt_qblk(b, i, lead):
                j = i + lead
                return (b + j // NBLK, j % NBLK)

            # software-pipelined schedule: q-pass(b) interleaved with kv(b+1);
            # loads prefetched 2 blocks ahead of their consumers.
            ctx_ps[0] = ps_c.tile([64, 64], F32, tag="ctx_ps", name="ctx_ps")
            load_q_block(0, 0)
            load_q_block(0, 1)
            load_kv_block(0, 0)
            load_kv_block(0, 1)
            for i in range(NBLK):
                if i + 2 < NBLK:
                    load_kv_block(0, i + 2)
                else:
                    load_kv_block(1, i + 2 - NBLK)
                emit_kv_block(0, i)
            ctxa = emit_ctx_epilogue(0)
            for b in range(BPC):
                if b + 1 < BPC:
                    ctx_ps[b + 1] = ps_c.tile([64, 64], F32, tag="ctx_ps", name="ctx_ps")
                nxt = None
                for i in range(NBLK):
                    # kv(b+1) first so its ctx completes before q(b) drains;
                    # epilogue right after the last kv block
                    if b + 1 < BPC:
                        kb, ki = b + 1, i + 2
                        if ki >= NBLK:
                            kb, ki = b + 2, ki - NBLK
                        if kb < BPC:
                            load_kv_block(kb, ki)
                        emit_kv_block(b + 1, i)
                        if i == NBLK - 1:
                            nxt = emit_ctx_epilogue(b + 1)
                    qb, qi = nxt_qblk(b, i, 2)
                    if qb < BPC:
                        load_q_block(qb, qi)
                    last = b == BPC - 1 and i == NBLK - 1
                    emit_q_block(b, i, ctxa, split_store=last)
                if nxt is not None:
                    ctxa = nxt

    nc.compile()
    return nc


_NC_CACHE = None


def kernel(q: np.ndarray, k: np.ndarray, v: np.ndarray) -> np.ndarray:
    global _NC_CACHE
    if _NC_CACHE is None:
        _NC_CACHE = build_bass()
    nc = _NC_CACHE
    q = np.ascontiguousarray(np.asarray(q), dtype=np.float16)
    k = np.ascontiguousarray(np.asarray(k), dtype=np.float16)
    v = np.ascontiguousarray(np.asarray(v), dtype=np.float16)
    in_maps = [
        {
            "q": q[i * BPC : (i + 1) * BPC],
            "k": k[i * BPC : (i + 1) * BPC],
            "v": v[i * BPC : (i + 1) * BPC],
        }
        for i in range(NCORES)
    ]
    res = run_bass_kernel_spmd(nc, in_maps, core_ids=list(range(NCORES)))
    return np.concatenate(
        [res.results[i]["o"] for i in range(NCORES)], axis=0
    ).astype(np.float32)
